# revision 1
# baseline (speedup 1.0000x reference)
"""Trainium2 Bass kernel for the triplet exp-distance loss.

loss = mean_i[ D_ap*(D_ap - v_ap)^2 + D_an*(D_an - v_an)^2 ]
  D_xx = exp(-triplets_dis[batch_index][:, k])
  v_xx = exp(-||a - x||_2)

Strategy: pure data parallel over 8 NeuronCores (65536 rows each).
Per core, SBUF partition p owns 512 contiguous rows; the shard streams
in 16 chunks of [128 part x 32 rows x 128 dim] f32 via 2MB HWDGE DMAs
(16KB contiguous per partition).  Per chunk: DVE computes diff
(f32 -> bf16), ACT squares in place (bf16), DVE tree-adds + reduces to
per-row squared norms.  A single batched tail does sqrt / exp and the
weighted squared error, accumulating into a [128, 2] partial that the
host sums across cores.
"""

import numpy as np

import concourse.bass as bass
import concourse.mybir as mb
import concourse.tile as tile
from concourse.bass_utils import run_bass_kernel_spmd

B = 524288
D = 128
M = 8                 # cores
S = B // M            # rows per core = 65536
P = 128               # SBUF partitions
RPP = S // P          # rows per partition = 512
C = 16                # rows per partition per chunk
NCH = RPP // C        # chunks
FD = C * D            # free-dim elements per chunk
IO_BUFS = 3           # input-tile double/triple buffering
GPSIMD_SUB = False    # offload one subtract per pair to GPSIMD

F32 = mb.dt.float32
BF16 = mb.dt.bfloat16


def _split_multiwaits(nc):
    """This walrus build accepts only one sync-wait per instruction.
    Hoist extra waits onto standalone single-wait InstEventSemaphore
    instructions inserted just before, on the same engine (semantically
    identical: the engine queue blocks on each in sequence)."""
    n_split = 0
    for f in nc.m.functions:
        for bb in f.blocks:
            insts = bb.instructions
            out = []
            changed = False
            for ins in insts:
                si = getattr(ins, "sync_info", None)
                if si is not None and si.on_wait is not None and len(si.on_wait) > 1:
                    waits = list(si.on_wait)
                    for k, w in enumerate(waits[:-1]):
                        ev = mb.InstEventSemaphore(
                            name=f"{ins.name}-wsplit{k}",
                            engine=ins.engine,
                            ins=[],
                            outs=[],
                            sync_info=mb.SyncInfo(on_wait=[w], on_update=[]),
                        )
                        out.append(ev)
                        n_split += 1
                    si.on_wait.clear()
                    si.on_wait.append(waits[-1])
                    changed = True
                out.append(ins)
            if changed:
                bb.instructions = out
    return n_split


def _build():
    nc = bass.Bass(trn_type="TRN2", name="triplet_loss")
    a = nc.dram_tensor("a", [S, D], F32, kind="ExternalInput")
    p = nc.dram_tensor("p", [S, D], F32, kind="ExternalInput")
    n = nc.dram_tensor("n", [S, D], F32, kind="ExternalInput")
    td = nc.dram_tensor("td", [S, 2], F32, kind="ExternalInput")
    out = nc.dram_tensor("out", [P, 4], F32, kind="ExternalOutput")

    # Partition p owns rows [p*RPP, (p+1)*RPP): contiguous per-partition
    # DRAM runs -> ideal DMA descriptors (16KB contiguous each).
    av = a.rearrange("(p n) d -> p (n d)", p=P)    # [128, RPP*D]
    pv = p.rearrange("(p n) d -> p (n d)", p=P)
    nv = n.rearrange("(p n) d -> p (n d)", p=P)
    tdv = td.rearrange("(p n) t -> p n t", p=P)    # [128, RPP, 2]

    with tile.TileContext(nc) as tc:
        with tc.tile_pool(name="io", bufs=IO_BUFS) as io, \
             tc.tile_pool(name="wk", bufs=2) as wk, \
             tc.tile_pool(name="tl", bufs=2) as tl, \
             tc.tile_pool(name="res", bufs=1) as res:
            td_t = res.tile([P, RPP, 2], F32)
            nc.sync.dma_start(out=td_t, in_=tdv)

            n2 = {}
            n2["p"] = res.tile([P, RPP], F32, tag="n2p", name="n2p")
            n2["n"] = res.tile([P, RPP], F32, tag="n2n", name="n2n")

            for c in range(NCH):
                sl = slice(c * FD, (c + 1) * FD)
                at = io.tile([P, FD], F32, tag="a")
                nc.sync.dma_start(out=at, in_=av[:, sl])
                pt = io.tile([P, FD], F32, tag="p")
                nc.sync.dma_start(out=pt, in_=pv[:, sl])
                nt = io.tile([P, FD], F32, tag="n")
                nc.sync.dma_start(out=nt, in_=nv[:, sl])

                at3 = at.rearrange("p (c d) -> p c d", d=D)
                for key, ot in (("p", pt), ("n", nt)):
                    ot3 = ot.rearrange("p (c d) -> p c d", d=D)
                    df = wk.tile([P, C, D], BF16, tag="d" + key)
                    # split the two subtracts across DVE and GPSIMD so the
                    # 1x f32 passes don't pile onto one engine
                    sub_eng = nc.gpsimd if (GPSIMD_SUB and key == "p") else nc.vector
                    sub_eng.tensor_sub(out=df, in0=at3, in1=ot3)
                    # square in place on ACT (bf16, 1x)
                    nc.scalar.activation(
                        out=df, in_=df, func=mb.ActivationFunctionType.Square
                    )
                    # bf16 2x tree adds, then 1x reduce of the last quarter
                    h1 = wk.tile([P, C, D // 2], BF16, tag="h1" + key)
                    nc.vector.tensor_add(
                        out=h1, in0=df[:, :, 0 : D // 2], in1=df[:, :, D // 2 : D]
                    )
                    h2 = wk.tile([P, C, D // 4], BF16, tag="h2" + key)
                    nc.vector.tensor_add(
                        out=h2, in0=h1[:, :, 0 : D // 4], in1=h1[:, :, D // 4 : D // 2]
                    )
                    nc.vector.reduce_sum(
                        out=n2[key][:, c * C : (c + 1) * C],
                        in_=h2,
                        axis=mb.AxisListType.X,
                    )

            # ---- batched tail, in halves so half overlaps the chunk loop ----
            dex = res.tile([P, RPP, 2], F32)
            acc = res.tile([P, 2, 2], F32)   # [P, half, pair]

            def tail(h):
                rs = slice(h * (RPP // 2), (h + 1) * (RPP // 2))
                for key in ("p", "n"):
                    nv_ = n2[key][:, rs]
                    nc.scalar.activation(out=nv_, in_=nv_, func=mb.ActivationFunctionType.Sqrt)
                nc.scalar.activation(out=dex[:, rs, :], in_=td_t[:, rs, :],
                                     func=mb.ActivationFunctionType.Exp, scale=-1.0)
                for i, key in enumerate(("p", "n")):
                    nv_ = n2[key][:, rs]
                    nc.scalar.activation(out=nv_, in_=nv_, func=mb.ActivationFunctionType.Exp, scale=-1.0)
                    dcol = dex[:, rs, i]
                    t_ = tl.tile([P, RPP // 2], F32, tag="t")
                    nc.vector.tensor_sub(out=t_, in0=dcol, in1=nv_)
                    m_ = tl.tile([P, RPP // 2], F32, tag="m")
                    nc.vector.tensor_mul(out=m_, in0=dcol, in1=t_)
                    sc = tl.tile([P, RPP // 2], F32, tag="sc")
                    nc.vector.tensor_mul(out=sc, in0=m_, in1=t_)
                    nc.vector.reduce_sum(
                        out=acc[:, h, i : i + 1], in_=sc, axis=mb.AxisListType.X
                    )

            tail(0)
            tail(1)
            nc.sync.dma_start(out=out[:, :], in_=acc.rearrange('p h i -> p (h i)'))

    _split_multiwaits(nc)
    return nc


_CACHE = {}


def _get_nc():
    if "nc" not in _CACHE:
        _CACHE["nc"] = _build()
    return _CACHE["nc"]


def _run(inputs, **spmd_kwargs):
    a = np.asarray(inputs["embedding_a"], dtype=np.float32)
    p = np.asarray(inputs["embedding_p"], dtype=np.float32)
    n = np.asarray(inputs["embedding_n"], dtype=np.float32)
    tdis = np.asarray(inputs["triplets_dis"], dtype=np.float32)
    bidx = np.asarray(inputs["batch_index"])
    td = np.ascontiguousarray(tdis[bidx])

    in_maps = [
        {
            "a": a[i * S : (i + 1) * S],
            "p": p[i * S : (i + 1) * S],
            "n": n[i * S : (i + 1) * S],
            "td": td[i * S : (i + 1) * S],
        }
        for i in range(M)
    ]
    r = run_bass_kernel_spmd(_get_nc(), in_maps, core_ids=list(range(M)), **spmd_kwargs)
    total = sum(res["out"].astype(np.float64).sum() for res in r.results)
    return np.float32(total / B), r


def kernel(**inputs):
    loss, _ = _run(inputs)
    return loss



# revision 2
# speedup vs baseline: 1.6350x; 1.6350x over previous
"""Trainium2 Bass kernel for the triplet exp-distance loss.

loss = mean_i[ D_ap*(D_ap - v_ap)^2 + D_an*(D_an - v_an)^2 ]
  D_xx = exp(-triplets_dis[batch_index][:, k])
  v_xx = exp(-||a - x||_2)

Pure data parallel over 8 NeuronCores (65536 rows each; SBUF partition p
owns 512 contiguous rows).  The dominant cost is streaming the three
[65536, 128] f32 embeddings; this kernel halves that HBM->SBUF charge by
casting f32->bf16 during the DMA (SWDGE / gpsimd path), which also gives
2x DVE throughput downstream:

  per 32-row chunk:  ta, tp, tn <= a, p, n   (cast DMAs, 3-chunk lookahead)
                     tp <= ta - tp ; tn <= ta - tn      [DVE, bf16 2x]
                     square in place                    [ACT]
                     3-level bf16 halving tree + reduce [DVE] -> |d|^2

exp(-td) for all rows is computed once, early.  The sqrt/exp/weighted-
error tail runs in row-slices interleaved with the chunk loop, the last
rows are processed in shrinking chunks (16/12/4), and results ship in
two DMAs so almost nothing remains after the last input transfer.
"""

import numpy as np

import concourse.bass as bass
import concourse.mybir as mb
import concourse.tile as tile
from concourse.bass_utils import run_bass_kernel_spmd

B = 524288
D = 128
M = 8                 # cores
S = B // M            # rows per core = 65536
P = 128               # SBUF partitions
RPP = S // P          # rows per partition = 512

F32 = mb.dt.float32
BF16 = mb.dt.bfloat16
AF = mb.ActivationFunctionType

# chunk schedule (row0, rows): bulk C=32, shrinking epilogue
CHUNKS = [(i * 32, 32) for i in range(15)] + [(480, 16), (496, 12), (508, 4)]
# tail slices: (row0, rows, after_chunk_index)
TAIL_SLICES = [
    (0, 128, 3),
    (128, 128, 7),
    (256, 128, 11),
    (384, 96, 14),
    (480, 16, 15),
    (496, 16, 17),
]
OUT_SPLIT = 5         # first 5 slice results ship mid-stream
LOOKAHEAD = 3         # chunk-load lookahead
IO_BUFS = 5
TREE_LEVELS = 3


def _split_multiwaits(nc, dummy_sem):
    """This walrus build accepts only one sync-wait per instruction.
    Hoist extra waits onto standalone single-wait InstEventSemaphore
    instructions inserted just before, on the same engine.  Each split
    event increments a dummy semaphore so the race detector's
    every-instruction-has-updates invariant holds."""
    import bass_rust as _bass_rust
    n_split = 0
    for f in nc.m.functions:
        for bb in f.blocks:
            insts = bb.instructions
            out = []
            changed = False
            for ins in insts:
                si = getattr(ins, "sync_info", None)
                if si is not None and si.on_wait is not None and len(si.on_wait) > 1:
                    waits = list(si.on_wait)
                    for k, w in enumerate(waits[:-1]):
                        ev = mb.InstEventSemaphore(
                            name=f"{ins.name}-wsplit{k}",
                            engine=ins.engine,
                            ins=[],
                            outs=[],
                            sync_info=mb.SyncInfo(on_wait=[w], on_update=[]),
                        )
                        _bass_rust.then_inc(ev, dummy_sem, 1, False)
                        out.append(ev)
                        n_split += 1
                    si.on_wait.clear()
                    si.on_wait.append(waits[-1])
                    changed = True
                out.append(ins)
            if changed:
                bb.instructions = out
    return n_split


def _build():
    nc = bass.Bass(trn_type="TRN2", name="triplet_loss")
    wsplit_sem = nc.alloc_semaphore("wsplit_dummy")
    a = nc.dram_tensor("a", [S, D], F32, kind="ExternalInput")
    p = nc.dram_tensor("p", [S, D], F32, kind="ExternalInput")
    n = nc.dram_tensor("n", [S, D], F32, kind="ExternalInput")
    td = nc.dram_tensor("td", [S, 2], F32, kind="ExternalInput")
    n_slices = len(TAIL_SLICES)
    out0 = nc.dram_tensor("out0", [P, OUT_SPLIT * 2], F32, kind="ExternalOutput")
    out1 = nc.dram_tensor("out1", [P, (n_slices - OUT_SPLIT) * 2], F32,
                          kind="ExternalOutput")

    av = a.rearrange("(p n) d -> p (n d)", p=P)    # [128, RPP*D]
    pv = p.rearrange("(p n) d -> p (n d)", p=P)
    nv = n.rearrange("(p n) d -> p (n d)", p=P)
    tdv = td.rearrange("(p n) t -> p n t", p=P)    # [128, RPP, 2]

    slices_by_chunk = {}
    for (r0, rows, after) in TAIL_SLICES:
        slices_by_chunk.setdefault(after, []).append((r0, rows))

    with tile.TileContext(nc) as tc:
        with tc.tile_pool(name="io", bufs=IO_BUFS) as io, \
             tc.tile_pool(name="wk", bufs=2) as wk, \
             tc.tile_pool(name="tl", bufs=2) as tl, \
             tc.tile_pool(name="res", bufs=1) as res:
            td_t = res.tile([P, RPP, 2], F32)
            n2 = {
                "p": res.tile([P, RPP], F32, tag="n2p", name="n2p"),
                "n": res.tile([P, RPP], F32, tag="n2n", name="n2n"),
            }
            dex = res.tile([P, RPP, 2], F32)
            acc = res.tile([P, n_slices, 2], F32)
            emitted = [0]
            tiles = {}

            def tail_slice(si, r0, rows):
                rs = slice(r0, r0 + rows)
                for key in ("p", "n"):
                    nv_ = n2[key][:, rs]
                    nc.scalar.activation(out=nv_, in_=nv_, func=AF.Sqrt)
                for i, key in enumerate(("p", "n")):
                    nv_ = n2[key][:, rs]
                    nc.scalar.activation(out=nv_, in_=nv_, func=AF.Exp, scale=-1.0)
                    dcol = dex[:, rs, i]
                    t_ = tl.tile([P, rows], F32, tag="t")
                    nc.vector.tensor_sub(out=t_, in0=dcol, in1=nv_)
                    m_ = tl.tile([P, rows], F32, tag="m")
                    nc.vector.tensor_mul(out=m_, in0=dcol, in1=t_)
                    sc = tl.tile([P, rows], F32, tag="sc")
                    nc.vector.tensor_mul(out=sc, in0=m_, in1=t_)
                    nc.vector.reduce_sum(
                        out=acc[:, si, i : i + 1], in_=sc, axis=mb.AxisListType.X
                    )

            def load(ci):
                r0, rows = CHUNKS[ci]
                sl = slice(r0 * D, (r0 + rows) * D)
                ts = []
                for nm, v in (("a", av), ("p", pv), ("n", nv)):
                    t = io.tile([P, rows * D], BF16, tag="t" + nm)
                    nc.gpsimd.dma_start(out=t, in_=v[:, sl])
                    ts.append(t)
                tiles[ci] = ts

            def compute(ci):
                r0, rows = CHUNKS[ci]
                ta, tp, tn = tiles.pop(ci)
                nc.vector.tensor_sub(out=tp, in0=ta, in1=tp)
                nc.vector.tensor_sub(out=tn, in0=ta, in1=tn)
                for key, t in (("p", tp), ("n", tn)):
                    nc.scalar.activation(out=t, in_=t, func=AF.Square)
                    h = t.rearrange("p (c d) -> p c d", d=D)
                    w = D
                    for lv in range(TREE_LEVELS):
                        hn = wk.tile([P, rows, w // 2], BF16, tag=f"h{lv}" + key)
                        nc.vector.tensor_add(
                            out=hn, in0=h[:, :, 0:w // 2], in1=h[:, :, w // 2:w])
                        h, w = hn, w // 2
                    nc.vector.reduce_sum(
                        out=n2[key][:, r0:r0 + rows], in_=h, axis=mb.AxisListType.X
                    )
                for (sr0, srows) in slices_by_chunk.get(ci, []):
                    tail_slice(emitted[0], sr0, srows)
                    emitted[0] += 1
                    if emitted[0] == OUT_SPLIT:
                        nc.sync.dma_start(
                            out=out0[:, :],
                            in_=acc[:, 0:OUT_SPLIT, :].rearrange("p s i -> p (s i)"))

            nch = len(CHUNKS)
            for ci in range(min(LOOKAHEAD, nch)):
                load(ci)
                if ci == 1:
                    nc.sync.dma_start(out=td_t, in_=tdv)
                    nc.scalar.activation(
                        out=dex.rearrange("p n t -> p (n t)"),
                        in_=td_t.rearrange("p n t -> p (n t)"),
                        func=AF.Exp, scale=-1.0)
            for ci in range(nch):
                if ci + LOOKAHEAD < nch:
                    load(ci + LOOKAHEAD)
                compute(ci)
            assert emitted[0] == n_slices
            nc.sync.dma_start(
                out=out1[:, :],
                in_=acc[:, OUT_SPLIT:n_slices, :].rearrange("p s i -> p (s i)"))

    _split_multiwaits(nc, wsplit_sem)
    return nc


_CACHE = {}


def _get_nc():
    if "nc" not in _CACHE:
        _CACHE["nc"] = _build()
    return _CACHE["nc"]


def _run(inputs, **spmd_kwargs):
    a = np.asarray(inputs["embedding_a"], dtype=np.float32)
    p = np.asarray(inputs["embedding_p"], dtype=np.float32)
    n = np.asarray(inputs["embedding_n"], dtype=np.float32)
    tdis = np.asarray(inputs["triplets_dis"], dtype=np.float32)
    bidx = np.asarray(inputs["batch_index"])
    td = np.ascontiguousarray(tdis[bidx])

    in_maps = [
        {
            "a": a[i * S : (i + 1) * S],
            "p": p[i * S : (i + 1) * S],
            "n": n[i * S : (i + 1) * S],
            "td": td[i * S : (i + 1) * S],
        }
        for i in range(M)
    ]
    r = run_bass_kernel_spmd(_get_nc(), in_maps, core_ids=list(range(M)), **spmd_kwargs)
    total = sum(
        res["out0"].astype(np.float64).sum() + res["out1"].astype(np.float64).sum()
        for res in r.results
    )
    return np.float32(total / B), r


def kernel(**inputs):
    loss, _ = _run(inputs)
    return loss
